# revision 13
# baseline (speedup 1.0000x reference)
"""Causal self-attention with ALiBi on 8 trn2 cores.

Sharding: data-parallel over batch (2) x tensor-parallel over head groups (4).
Core c handles batch b = c // 4, head group g = c % 4 (heads 4g..4g+3).
Each core computes qT/kT/v projections for its 4 heads, flash-style causal
attention with ALiBi folded into the score matmul via 2 augmented K rows
(k_aug = [iota_j; ones], q_aug = [slope; -slope*i]), and a partial output
projection.  Host sums the 4 partials per batch and adds bo.

All matmuls run in float32r (~13-bit mantissa, 4x faster than fp32 on PE).
The ALiBi term is exact in f32r: j and slope are exactly representable, and
the -slope*i term is constant per query so softmax cancels its rounding.
"""

import sys

sys.path.insert(0, "/opt/trn_rl_repo")

import numpy as np

import concourse.bacc as bacc
import concourse.mybir as mybir
import concourse.tile as tile
from concourse.bass import ds, ts
from concourse.bass_utils import run_bass_kernel_spmd

B, T, D, H, DH = 2, 2048, 1024, 16, 64
G = 4            # head groups (tensor-parallel)
HPC = H // G     # heads per core
DG = D // G      # model dims per core (256)
P = 128
N_CORES = 8
NEG = -1.0e30

F32 = mybir.dt.float32
F32R = mybir.dt.float32r
BF16 = mybir.dt.bfloat16
ADD = mybir.AluOpType.add
MULT = mybir.AluOpType.mult
EXP = mybir.ActivationFunctionType.Exp

TRACE = False
LAST_RESULTS = None

_cache = {}


def _build(with_bias: bool):
    nc = bacc.Bacc("TRN2", target_bir_lowering=False, debug=False)

    xT_d = nc.dram_tensor("xT", [D, T], F32, kind="ExternalInput").ap()
    wq_d = nc.dram_tensor("wqT", [D, DG], F32, kind="ExternalInput").ap()
    wk_d = nc.dram_tensor("wkT", [D, DG], F32, kind="ExternalInput").ap()
    wv_d = nc.dram_tensor("wvT", [D, HPC * 65], F32, kind="ExternalInput").ap()
    wo_d = nc.dram_tensor("woT", [DG, D], F32, kind="ExternalInput").ap()
    qaug_d = nc.dram_tensor("qaug", [HPC, 2, T], F32, kind="ExternalInput").ap()
    kaug_d = nc.dram_tensor("kaug", [2, T], F32, kind="ExternalInput").ap()
    ident_d = nc.dram_tensor("ident", [P, P], BF16, kind="ExternalInput").ap()
    maskst_d = nc.dram_tensor("maskst", [P, P], BF16, kind="ExternalInput").ap()
    bvo_d = nc.dram_tensor("bv_ones", [P, HPC * 65], F32, kind="ExternalInput").ap()
    if with_bias:
        bq_d = nc.dram_tensor("bq2", [P, 2], F32, kind="ExternalInput").ap()
        bk_d = nc.dram_tensor("bk2", [P, 2], F32, kind="ExternalInput").ap()
    out_d = nc.dram_tensor("outT", [D, T], F32, kind="ExternalOutput").ap()
    dscr_d = nc.dram_tensor("dscratch", [16, 512], F32).ap()

    with tile.TileContext(nc) as tc:
        with (
            tc.tile_pool(name="big", bufs=1) as big,
            tc.tile_pool(name="stage", bufs=3) as stage,
            tc.tile_pool(name="expp", bufs=4) as expp,
            tc.tile_pool(name="small", bufs=1) as small,
            tc.tile_pool(name="ps512", bufs=2, space="PSUM") as ps512,
            tc.tile_pool(name="pss", bufs=2, space="PSUM") as pssp,
            tc.tile_pool(name="psy", bufs=4, space="PSUM") as psyp,
        ):
            # ---- constants / weights / inputs into SBUF
            xt = []
            for i in range(8):
                t_ = big.tile([P, T], F32R, tag=f"xt{i}", name=f"xt{i}")
                nc.sync.dma_start(out=t_[:], in_=xT_d[ts(i, P), :].bitcast(F32R))
                xt.append(t_)
            wq, wk, wv = [], [], []
            for i in range(8):
                t_ = big.tile([P, DG], F32R, tag=f"wq{i}", name=f"wq{i}")
                nc.sync.dma_start(out=t_[:], in_=wq_d[ts(i, P), :].bitcast(F32R))
                wq.append(t_)
                t_ = big.tile([P, DG], F32R, tag=f"wk{i}", name=f"wk{i}")
                nc.sync.dma_start(out=t_[:], in_=wk_d[ts(i, P), :].bitcast(F32R))
                wk.append(t_)
                t_ = big.tile([P, HPC * 65], F32R, tag=f"wv{i}", name=f"wv{i}")
                nc.sync.dma_start(out=t_[:], in_=wv_d[ts(i, P), :].bitcast(F32R))
                wv.append(t_)
            ident_sb = big.tile([P, P], BF16, tag="ident")
            nc.sync.dma_start(out=ident_sb[:], in_=ident_d[:])
            maskst_sb = big.tile([P, P], BF16, tag="maskst")
            nc.sync.dma_start(out=maskst_sb[:], in_=maskst_d[:])
            bvo = big.tile([P, HPC * 65], F32, tag="bvo")
            nc.sync.dma_start(out=bvo[:], in_=bvo_d[:])
            if with_bias:
                bq2 = big.tile([P, 2], F32, tag="bq2")
                nc.sync.dma_start(out=bq2[:], in_=bq_d[:])
                bk2 = big.tile([P, 2], F32, tag="bk2")
                nc.sync.dma_start(out=bk2[:], in_=bk_d[:])

            # head tiles: rows 0..63 data, rows 64..65 [row0;row1] aug pairs
            qa = [big.tile([P, T], F32R, tag=f"qa{h}", name=f"qa{h}") for h in range(HPC)]
            ka = [big.tile([P, T], F32R, tag=f"ka{h}", name=f"ka{h}") for h in range(HPC)]
            va = [big.tile([P, 16, P], F32R, tag=f"va{h}", name=f"va{h}") for h in range(HPC)]
            for h in range(HPC):
                nc.sync.dma_start(out=qa[h][64:66, :], in_=qaug_d[h].bitcast(F32R))
                nc.sync.dma_start(out=ka[h][64:66, :], in_=kaug_d[:].bitcast(F32R))
                # ones column for the in-matmul softmax denominator; the odd
                # head's lands at partition 32 (engine APs need 32-aligned base)
                oc = 64 if h % 2 == 0 else 32
                for ch in range(16):
                    nc.vector.memset(va[h][:, ch, oc : oc + 1].bitcast(F32), 1.0)
            yt = [big.tile([P, T], F32R, tag=f"xt{m}", name=f"yt{m}") for m in range(2)]

            # ---- v projection (natural [t, d] layout + ones/bias columns)
            for ch in range(16):
                pv = psyp.tile([P, HPC * 65], F32, tag="psy", name="pv")
                for kc in range(8):
                    nc.tensor.matmul(
                        out=pv[:],
                        lhsT=xt[kc][:, ts(ch, P)],
                        rhs=wv[kc][:],
                        start=(kc == 0),
                        stop=(kc == 7),
                    )
                for h in range(HPC):
                    off = 0 if h % 2 == 0 else 64
                    nc.vector.tensor_tensor(
                        out=va[h][:, ch, off : off + 64],
                        in0=pv[:, h * 65 : h * 65 + 64],
                        in1=bvo[:, h * 65 : h * 65 + 64],
                        op=ADD,
                    )

            # ---- q/k projections into [d', t] layout, split per head
            for wt, dst, bias_name in ((wq, qa, "q"), (wk, ka, "k")):
                bt = (bq2 if bias_name == "q" else bk2) if with_bias else None
                for mc in range(2):
                    for nt in range(4):
                        pq = ps512.tile([P, 512], F32, tag="mm512")
                        for kc in range(8):
                            nc.tensor.matmul(
                                out=pq[:],
                                lhsT=wt[kc][:, ts(mc, P)],
                                rhs=xt[kc][:, ts(nt, 512)],
                                start=(kc == 0),
                                stop=(kc == 7),
                            )
                        h_even, h_odd = 2 * mc, 2 * mc + 1
                        if with_bias:
                            nc.vector.tensor_scalar(
                                out=dst[h_even][0:64, ts(nt, 512)],
                                in0=pq[0:64, :],
                                scalar1=bt[0:64, mc : mc + 1],
                                scalar2=None,
                                op0=ADD,
                            )
                        else:
                            nc.vector.tensor_copy(
                                out=dst[h_even][0:64, ts(nt, 512)], in_=pq[0:64, :]
                            )
                        st = stage.tile([P, 512], F32R, tag="stage")
                        if with_bias:
                            nc.vector.tensor_scalar(
                                out=st[64:128, :],
                                in0=pq[64:128, :],
                                scalar1=bt[64:128, mc : mc + 1],
                                scalar2=None,
                                op0=ADD,
                            )
                        else:
                            nc.vector.tensor_copy(out=st[64:128, :], in_=pq[64:128, :])
                        nc.sync.dma_start(
                            out=dst[h_odd][0:64, ts(nt, 512)], in_=st[64:128, :]
                        )

            # ---- attention: causal flash over j-chunks of 128, q-blocks of 512
            # wo loaded up front (slots free after k-proj); out-proj for each
            # q-block is emitted right after its normalize to overlap with the
            # next q-block's attention.
            wo = []
            for i in range(2):
                t_ = big.tile([P, D], F32R, tag=f"xt{i + 2}", name=f"wo{i}")
                nc.sync.dma_start(out=t_[:], in_=wo_d[ts(i, P), :].bitcast(F32R))
                wo.append(t_)
            for qb in range(4):
                o = qb * 512
                jmax = qb * 4 + 4
                for pair in range(2):
                    pys = []
                    for h in (2 * pair, 2 * pair + 1):
                        py = psyp.tile([P, 512], F32, tag="psy", name=f"py{qb}_{h}")
                        pys.append(py)
                        pend = None  # software-pipeline: emit AV one chunk behind
                        for jc in range(jmax):
                            r = jc * P - o  # stair offset; diag chunk iff r >= 0
                            ps = pssp.tile([P, 512], F32, tag="pss", name=f"ps{qb}_{h}_{jc}")
                            if r < 0:
                                nc.tensor.matmul(
                                    out=ps[:],
                                    lhsT=ka[h][0:66, ts(jc, P)],
                                    rhs=qa[h][0:66, ds(o, 512)],
                                    start=True,
                                    stop=True,
                                )
                                lo = 0
                            else:
                                lo = r
                                nc.tensor.matmul(
                                    out=ps[:, lo:512],
                                    lhsT=ka[h][0:66, ts(jc, P)],
                                    rhs=qa[h][0:66, ds(o + lo, 512 - lo)],
                                    start=True,
                                    stop=False,
                                )
                                # causal stair: ps[:, r:r+128] += I.T @ maskst
                                nc.tensor.matmul(
                                    out=ps[:, lo : lo + P],
                                    lhsT=ident_sb[:],
                                    rhs=maskst_sb[:],
                                    start=False,
                                    stop=True,
                                )
                            ex = expp.tile([P, 512], F32R, tag="ex", name=f"ex{qb}_{h}_{jc}")
                            nc.scalar.activation(out=ex[:, lo:512], in_=ps[:, lo:512], func=EXP)
                            if pend is not None:
                                pjc, plo, pex = pend
                                nc.tensor.matmul(
                                    out=py[:, plo:512],
                                    lhsT=va[h][:, pjc, :],
                                    rhs=pex[:, plo:512],
                                    start=(pjc == 0),
                                    stop=False,
                                )
                            pend = (jc, lo, ex)
                        pjc, plo, pex = pend
                        nc.tensor.matmul(
                            out=py[:, plo:512],
                            lhsT=va[h][:, pjc, :],
                            rhs=pex[:, plo:512],
                            start=(pjc == 0),
                            stop=True,
                        )
                    # pair-batched softmax denominators at 32-aligned rows
                    dn = small.tile([P, 512], F32, tag="dn", name=f"dn{qb}_{pair}")
                    for i in range(2):
                        dr = 64 if i == 0 else 32
                        nc.vector.tensor_copy(
                            out=dn[i * 32 : i * 32 + 1, :], in_=pys[i][dr : dr + 1, :]
                        )
                    dn2 = small.tile([P, 512], F32, tag="dn2", name=f"dn2{qb}_{pair}")
                    nc.vector.reciprocal(out=dn2[0:64, :], in_=dn[0:64, :])
                    for i in range(2):
                        h = 2 * pair + i
                        rowbase = i * 64
                        idx = qb * HPC + h
                        nc.sync.dma_start(
                            out=dscr_d[idx : idx + 1, :], in_=dn2[i * 32 : i * 32 + 1, :]
                        )
                        rb = small.tile([P, 512], F32, tag="rb", name=f"rb{qb}_{h}")
                        nc.sync.dma_start(
                            out=rb[rowbase : rowbase + 64, :],
                            in_=dscr_d[idx : idx + 1, :].to_broadcast((64, 512)),
                        )
                        nc.vector.tensor_tensor(
                            out=yt[pair][rowbase : rowbase + 64, ds(o, 512)],
                            in0=pys[i][rowbase : rowbase + 64, :],
                            in1=rb[rowbase : rowbase + 64, :],
                            op=MULT,
                        )
                # ---- output projection for this q-block (partial over heads)
                for ec in range(8):
                    po = ps512.tile([P, 512], F32, tag="mm512", name=f"po{qb}_{ec}")
                    for k2 in range(2):
                        nc.tensor.matmul(
                            out=po[:],
                            lhsT=wo[k2][:, ts(ec, P)],
                            rhs=yt[k2][:, ts(qb, 512)],
                            start=(k2 == 0),
                            stop=(k2 == 1),
                        )
                    ob = stage.tile([P, 512], F32, tag="stage", name="ob")
                    nc.vector.tensor_copy(out=ob[:], in_=po[:])
                    nc.sync.dma_start(out=out_d[ts(ec, P), ts(qb, 512)], in_=ob[:])

    nc.compile()
    return nc


def _get_nc(with_bias: bool):
    if with_bias not in _cache:
        _cache[with_bias] = _build(with_bias)
    return _cache[with_bias]


def kernel(x, freqs_cis, Wq, bq, Wkv, bkv, Wo, bo, **_unused):
    x = np.asarray(x, np.float32)
    Wq = np.asarray(Wq, np.float32)
    bq = np.asarray(bq, np.float32)
    Wkv = np.asarray(Wkv, np.float32)
    bkv = np.asarray(bkv, np.float32)
    Wo = np.asarray(Wo, np.float32)
    bo = np.asarray(bo, np.float32)

    with_bias = bool(np.any(bq) or np.any(bkv))
    nc = _get_nc(with_bias)

    scale = 1.0 / np.sqrt(DH)
    iota = np.arange(T, dtype=np.float32)

    # causal stair (applied via identity-matmul accumulation into PSUM):
    # maskst[p, m] = -1e30 where m < p (j = chunk base + p is in the future)
    import ml_dtypes
    mm = np.arange(P, dtype=np.float32)
    maskst = np.where(mm[None, :] < mm[:, None], NEG, 0.0).astype(ml_dtypes.bfloat16)
    ident = np.eye(P, dtype=ml_dtypes.bfloat16)

    kaug = np.stack([iota, np.ones(T, np.float32)])  # [2, T]

    xT = [np.ascontiguousarray(x[b].T) for b in range(B)]  # [D, T]

    in_maps = []
    for c in range(N_CORES):
        b, g = divmod(c, G)
        rows = slice(g * DG, (g + 1) * DG)
        wqT = np.ascontiguousarray((Wq[rows] * scale).T)          # [D, DG]
        wkT = np.ascontiguousarray(Wkv[0:D][rows].T)              # [D, DG]
        wv_g = Wkv[D : 2 * D][rows]                               # [DG, D]
        bv_g = bkv[D : 2 * D][rows]                               # [DG]
        # v weights with one zero column per head: even head [v(64), 0], odd [0, v(64)]
        wvT = np.zeros((D, HPC * 65), np.float32)
        bvo = np.zeros((P, HPC * 65), np.float32)
        for h in range(HPC):
            wvT[:, h * 65 : h * 65 + 64] = wv_g[h * 64 : (h + 1) * 64].T
            bvo[:, h * 65 : h * 65 + 64] = bv_g[h * 64 : (h + 1) * 64][None, :]
        woT = np.ascontiguousarray(Wo[:, rows].T)                 # [DG, D]
        qaug = np.zeros((HPC, 2, T), np.float32)
        for h in range(HPC):
            slope = (g * HPC + h + 1) / H
            qaug[h, 0, :] = slope
            qaug[h, 1, :] = -slope * iota
        m = {
            "xT": xT[b],
            "wqT": wqT,
            "wkT": wkT,
            "wvT": wvT,
            "woT": woT,
            "qaug": qaug,
            "kaug": kaug,
            "ident": ident,
            "maskst": maskst,
            "bv_ones": bvo,
        }
        if with_bias:
            m["bq2"] = np.ascontiguousarray((bq[rows] * scale).reshape(2, P).T)
            m["bk2"] = np.ascontiguousarray(bkv[0:D][rows].reshape(2, P).T)
        in_maps.append(m)

    res = run_bass_kernel_spmd(nc, in_maps, list(range(N_CORES)), trace=TRACE)
    global LAST_RESULTS
    LAST_RESULTS = res

    out = np.empty((B, T, D), np.float32)
    for b in range(B):
        acc = res.results[b * G]["outT"].copy()
        for g in range(1, G):
            acc += res.results[b * G + g]["outT"]
        out[b] = acc.T + bo[None, :]
    return out
